# revision 3
# baseline (speedup 1.0000x reference)
"""LGA3 (3x local guided aggregation, radius 2) on 8 TRN2 NeuronCores.

Sharding: H split 8 ways (48 output rows/core), redundant-halo compute
(pass1 computes 56 rows, pass2 52, pass3 48) -> no inter-core comms.

v2 multi-engine design (vs v1 DVE-only fp32):
- bf16 data everywhere; PSUM accumulates in fp32 (exact).
- Per 8-row chunk and tap-group (g,i) of 5 j-taps:
    DVE (12 groups): q = window * w  -- one tensor_tensor in 2x perf mode
      (all operands bf16, innermost stride-1 = row dim -> needs the
      row-innermost "transposed" cost tile ct2 made by the ACT engine).
    Pool (3 groups): same on the row-major tile ct4 (no perf modes on
      Pool so strides don't matter).
    PE: 75 identity-stationary matmuls accumulate all q tiles into one
      PSUM bank (fp32, free).  ACT evicts PSUM -> bf16 slab (fp32 final).
- Layout: partition = x (128-chunks); cost slab [b, y, x, d] with x
  padded +-2 (772) and d padded +-1 (66) so every (j, g) shift is an
  affine free-dim offset; the 5 x-shifts are the j dim (stride 66).
  Weights pre-transposed on host to [b, x, t, y] (y innermost) with
  zeroed out-of-image rows.
"""

import os
import sys

for _p in ("/opt/trn_rl_repo", "/root/.axon_site/_ro/trn_rl_repo"):
    if os.path.isdir(_p) and _p not in sys.path:
        sys.path.append(_p)

import numpy as np
import concourse.bass as bass
import concourse.mybir as mybir
from concourse.tile import TileContext
from concourse import bass_utils, masks

F32 = mybir.dt.float32
BF16 = mybir.dt.bfloat16
NP_BF16 = mybir.dt.np(BF16)

B, D, H, W = 2, 64, 384, 768
N_CORES = 8
ROWS = H // N_CORES  # 48 output rows per core

# cost slab [b, 60 rows, 772 x, 66 d] (strides in elements)
S_X, S_ROW = 66, 772 * 66
S_B = 60 * S_ROW
# weight dram [b, 768 x, 75 t, 56 y]
W_B = 768 * 75 * 56
# output [b, 48 rows, 768 x, 64 d]
O_ROW, O_B = 768 * 64, 48 * 768 * 64

# chunk lists per pass: (slab row of first output row, n output rows)
CHUNKS = {
    1: [(2 + 8 * c, 8) for c in range(7)],                  # rows 2..57
    2: [(4 + 8 * c, 8) for c in range(6)] + [(52, 4)],      # rows 4..55
    3: [(6 + 8 * c, 8) for c in range(6)],                  # rows 6..53
}

GROUPS = [(g, i) for g in range(3) for i in range(5)]
POOL_GROUPS = [(0, 0), (1, 2), (2, 4)]
DVE_GROUPS = [gi for gi in GROUPS if gi not in POOL_GROUPS]

LAST_EXEC_NS = [None]


def _split_waits(nc, max_waits=1):
    """Split >max_waits sync waits on one instruction into preceding
    wait-only drains (walrus setupSyncWait limit workaround)."""
    ctr = [0]
    for f in nc.m.functions:
        for blk in f.blocks:
            new_list = []
            for inst in blk.instructions:
                si = getattr(inst, "sync_info", None)
                if si is not None and si.on_wait and len(si.on_wait) > max_waits:
                    waits = list(si.on_wait)
                    extra, keep = waits[:-max_waits], waits[-max_waits:]
                    for wcond in extra:
                        ctr[0] += 1
                        nop = mybir.InstDrain(
                            name=f"waitsplit_{ctr[0]}", ins=[], outs=[]
                        )
                        nop.engine = inst.engine
                        nop.sync_info = mybir.SyncInfo(on_wait=[wcond], on_update=[])
                        new_list.append(nop)
                        nc.register_instruction(nop, overwrite=True)
                    si.on_wait = keep
                new_list.append(inst)
            blk.instructions = new_list
    return nc


def _emit_pass(nc, src, wdram, dst, p):
    """One LGA pass: src slab -> dst (slab for p<3, compact output p=3)."""
    ev_dt = F32 if p == 3 else BF16
    with TileContext(nc) as tc:
        with (
            tc.tile_pool(name=f"w{p}", bufs=2) as wpool,
            tc.tile_pool(name=f"c4{p}", bufs=3) as c4pool,
            tc.tile_pool(name=f"c2{p}", bufs=3) as c2pool,
            tc.tile_pool(name=f"qd{p}", bufs=4) as qdpool,
            tc.tile_pool(name=f"qp{p}", bufs=3) as qppool,
            tc.tile_pool(name=f"ev{p}", bufs=3) as evpool,
            tc.tile_pool(name=f"id{p}", bufs=1) as idpool,
            tc.tile_pool(name=f"ps{p}", bufs=4, space="PSUM") as pspool,
        ):
            ident = idpool.tile([128, 128], BF16)
            masks.make_identity(nc, ident[:])
            for b in range(2):
                for xc in range(6):
                    wt = wpool.tile([128, 3, 5, 5, 56], BF16)
                    nc.sync.dma_start(
                        out=wt[:],
                        in_=bass.AP(
                            tensor=wdram,
                            offset=(b * 768 + xc * 128) * 75 * 56,
                            ap=[[75 * 56, 128], [25 * 56, 3], [5 * 56, 5], [56, 5], [1, 56]],
                        ),
                    )
                    for r0, nr in CHUNKS[p]:
                        r4 = nr + 4
                        y0 = r0 - 2
                        ct4 = c4pool.tile([128, r4, 5, 66], BF16)
                        nc.sync.dma_start(
                            out=ct4[:],
                            in_=bass.AP(
                                tensor=src,
                                offset=b * S_B + (r0 - 2) * S_ROW + xc * 128 * S_X,
                                ap=[[S_X, 128], [S_ROW, r4], [66, 5], [1, 66]],
                            ),
                        )
                        ct2 = c2pool.tile([128, 5, 66, r4], BF16)
                        nc.scalar.copy(ct2[:], ct4[:].rearrange("p r j s -> p j s r"))
                        psum = pspool.tile([128, nr, 64], F32)
                        n_mm = 0
                        for g, i in DVE_GROUPS:
                            q = qdpool.tile([128, 5, 64, nr], BF16)
                            nc.vector.tensor_mul(
                                out=q[:],
                                in0=ct2[:, :, g : g + 64, i : i + nr],
                                in1=wt[:, g, i, :, y0 : y0 + nr]
                                .unsqueeze(2)
                                .broadcast_to([128, 5, 64, nr]),
                            )
                            for j in range(5):
                                nc.tensor.matmul(
                                    psum[:],
                                    ident[:],
                                    q[:, j, :, :].rearrange("p d r -> p r d"),
                                    start=(n_mm == 0),
                                    stop=(n_mm == 74),
                                )
                                n_mm += 1
                        for g, i in POOL_GROUPS:
                            qp = qppool.tile([128, nr, 5, 64], BF16)
                            nc.gpsimd.tensor_mul(
                                out=qp[:],
                                in0=ct4[:, i : i + nr, :, g : g + 64],
                                in1=wt[:, g, i, :, y0 : y0 + nr]
                                .rearrange("p j r -> p r j")
                                .unsqueeze(3)
                                .broadcast_to([128, nr, 5, 64]),
                            )
                            for j in range(5):
                                nc.tensor.matmul(
                                    psum[:],
                                    ident[:],
                                    qp[:, :, j, :],
                                    start=(n_mm == 0),
                                    stop=(n_mm == 74),
                                )
                                n_mm += 1
                        ev = evpool.tile([128, nr, 64], ev_dt)
                        nc.scalar.copy(ev[:], psum[:])
                        if p < 3:
                            dst_ap = bass.AP(
                                tensor=dst,
                                offset=b * S_B
                                + r0 * S_ROW
                                + (xc * 128 + 2) * S_X
                                + 1,
                                ap=[[S_X, 128], [S_ROW, nr], [1, 64]],
                            )
                        else:
                            dst_ap = bass.AP(
                                tensor=dst,
                                offset=b * O_B + (r0 - 6) * O_ROW + xc * 128 * 64,
                                ap=[[64, 128], [O_ROW, nr], [1, 64]],
                            )
                        nc.sync.dma_start(out=dst_ap, in_=ev[:])


def _build():
    nc = bass.Bass()
    a = nc.dram_tensor("a", [2, 60, 772, 66], BF16, kind="ExternalInput")
    w = nc.dram_tensor("w", [2, 768, 75, 56], BF16, kind="ExternalInput")
    bs = nc.dram_tensor("bslab", [2, 60, 772, 66], BF16, kind="Internal")
    cs = nc.dram_tensor("cslab", [2, 60, 772, 66], BF16, kind="Internal")
    o = nc.dram_tensor("o", [2, 48, 768, 64], F32, kind="ExternalOutput")

    # ctx 0: zero both intermediate slabs (borders must read as zero pad).
    # The final 64 elements (b=1, row 59, x=771, d>=2) are never read, so
    # the //128 truncation on the last chunk is harmless.
    total = 2 * 60 * 772 * 66
    with TileContext(nc) as tc:
        with tc.tile_pool(name="z", bufs=1) as zp:
            zt = zp.tile([128, 4096], BF16)
            nc.vector.memset(zt[:], 0.0)
            for dstt in (bs, cs):
                off = 0
                while off < total:
                    n = min(128 * 4096, total - off)
                    cols = n // 128
                    nc.sync.dma_start(
                        out=bass.AP(
                            tensor=dstt, offset=off, ap=[[cols, 128], [1, cols]]
                        ),
                        in_=zt[:, :cols],
                    )
                    off += n

    _emit_pass(nc, a, w, bs, 1)
    _emit_pass(nc, bs, w, cs, 2)
    _emit_pass(nc, cs, w, o, 3)
    _split_waits(nc)
    return nc


def _prep_core(input1, input2, k):
    """Host-side shard prep for core k: bf16 padded cost slab + transposed
    weight slab (zeroed outside the image)."""
    s = k * ROWS
    slab = np.zeros((2, 60, 772, 66), NP_BF16)
    lo, hi = max(0, s - 6), min(H, s + 54)
    slab[:, lo - (s - 6) : hi - (s - 6), 2:770, 1:65] = (
        input1[:, :, lo:hi, :].transpose(0, 2, 3, 1).astype(NP_BF16)
    )
    wsl = np.zeros((2, 768, 75, 56), NP_BF16)
    lo2, hi2 = max(0, s - 4), min(H, s + 52)
    wsl[:, :, :, lo2 - (s - 4) : hi2 - (s - 4)] = (
        input2[:, :, lo2:hi2, :].transpose(0, 3, 1, 2).astype(NP_BF16)
    )
    return {"a": slab, "w": wsl}


_NC_CACHE = [None]


def kernel(input1: np.ndarray, input2: np.ndarray) -> np.ndarray:
    input1 = np.asarray(input1, dtype=np.float32)
    input2 = np.asarray(input2, dtype=np.float32)
    if _NC_CACHE[0] is None:
        _NC_CACHE[0] = _build()
    nc = _NC_CACHE[0]

    in_maps = [_prep_core(input1, input2, k) for k in range(N_CORES)]

    # trace=False: the axon NTFF profile hook (antenv.axon_hooks) is not
    # available in this container and trace=True fails before compiling.
    res = bass_utils.run_bass_kernel_spmd(
        nc, in_maps, core_ids=list(range(N_CORES)), trace=False
    )
    LAST_EXEC_NS[0] = res.exec_time_ns

    out = np.empty((B, D, H, W), np.float32)
    for k in range(N_CORES):
        s = k * ROWS
        out[:, :, s : s + ROWS, :] = res.results[k]["o"].transpose(0, 3, 1, 2)
    return out


# revision 10
# speedup vs baseline: 1.0193x; 1.0193x over previous
"""LGA3 (3x local guided aggregation, radius 2) on 8 TRN2 NeuronCores.

Sharding: H split 8 ways (48 output rows/core), redundant-halo compute
(pass1 computes 56 rows, pass2 52, pass3 48) -> no inter-core comms.

v2 multi-engine design (vs v1 DVE-only fp32):
- bf16 data everywhere; PSUM accumulates in fp32 (exact).
- Per 8-row chunk and tap-group (g,i) of 5 j-taps:
    DVE (12 groups): q = window * w  -- one tensor_tensor in 2x perf mode
      (all operands bf16, innermost stride-1 = row dim -> needs the
      row-innermost "transposed" cost tile ct2 made by the ACT engine).
    Pool (3 groups): same on the row-major tile ct4 (no perf modes on
      Pool so strides don't matter).
    PE: 75 identity-stationary matmuls accumulate all q tiles into one
      PSUM bank (fp32, free).  ACT evicts PSUM -> bf16 slab (fp32 final).
- Layout: partition = x (128-chunks); cost slab [b, y, x, d] with x
  padded +-2 (772) and d padded +-1 (66) so every (j, g) shift is an
  affine free-dim offset; the 5 x-shifts are the j dim (stride 66).
  Weights pre-transposed on host to [b, x, t, y] (y innermost) with
  zeroed out-of-image rows.
"""

import os
import sys

for _p in ("/opt/trn_rl_repo", "/root/.axon_site/_ro/trn_rl_repo"):
    if os.path.isdir(_p) and _p not in sys.path:
        sys.path.append(_p)

import numpy as np
import concourse.bass as bass
import concourse.mybir as mybir
from concourse.tile import TileContext
from concourse import bass_utils, masks

F32 = mybir.dt.float32
BF16 = mybir.dt.bfloat16
NP_BF16 = mybir.dt.np(BF16)

B, D, H, W = 2, 64, 384, 768
N_CORES = 8
ROWS = H // N_CORES  # 48 output rows per core

# cost slab [b, 60 rows, 772 x, 66 d] (strides in elements)
S_X, S_ROW = 66, 772 * 66
S_B = 60 * S_ROW
# weight dram [b, 768 x, 75 t, 56 y]
W_B = 768 * 75 * 56
# output [b, 48 rows, 768 x, 64 d]
O_ROW, O_B = 768 * 64, 48 * 768 * 64

# chunk lists per pass: (slab row of first output row, n output rows).
# 16-row blocks amortize per-instruction and per-chunk fixed costs; each
# block accumulates into ceil(nr/8) PSUM banks (a bank holds 8x64 fp32).
CHUNKS = {
    1: [(2, 16), (18, 16), (34, 16), (50, 8)],   # rows 2..57
    2: [(4, 16), (20, 16), (36, 16), (52, 4)],   # rows 4..55
    3: [(6, 16), (22, 16), (38, 16)],            # rows 6..53
}

GROUPS = [(g, i) for g in range(3) for i in range(5)]
POOL_GROUPS = [(0, 0), (1, 2), (2, 4)]
# DVE takes the other 12 (g,i) groups.  (Merging i-pairs into one
# instruction needs a 4th free dim, which the TensorTensor ISA pattern
# rejects: "Expected 3D tensor for ISA mem pattern".)
DVE_GROUPS = [gi for gi in GROUPS if gi not in POOL_GROUPS]

LAST_EXEC_NS = [None]


def _split_waits(nc, max_waits=1):
    """Split >max_waits sync waits on one instruction into preceding
    wait-only drains (walrus setupSyncWait limit workaround)."""
    ctr = [0]
    for f in nc.m.functions:
        for blk in f.blocks:
            new_list = []
            for inst in blk.instructions:
                si = getattr(inst, "sync_info", None)
                if si is not None and si.on_wait and len(si.on_wait) > max_waits:
                    waits = list(si.on_wait)
                    extra, keep = waits[:-max_waits], waits[-max_waits:]
                    for wcond in extra:
                        ctr[0] += 1
                        nop = mybir.InstDrain(
                            name=f"waitsplit_{ctr[0]}", ins=[], outs=[]
                        )
                        nop.engine = inst.engine
                        nop.sync_info = mybir.SyncInfo(on_wait=[wcond], on_update=[])
                        new_list.append(nop)
                        nc.register_instruction(nop, overwrite=True)
                    si.on_wait = keep
                new_list.append(inst)
            blk.instructions = new_list
    return nc


def _emit_pass(nc, src, wdram, dst, p):
    """One LGA pass: src slab -> dst (slab for p<3, compact output p=3)."""
    ev_dt = F32 if p == 3 else BF16
    with TileContext(nc) as tc:
        with (
            tc.tile_pool(name=f"w{p}", bufs=2) as wpool,
            tc.tile_pool(name=f"c4{p}", bufs=3) as c4pool,
            tc.tile_pool(name=f"c2{p}", bufs=3) as c2pool,
            tc.tile_pool(name=f"qd{p}", bufs=4) as qdpool,
            tc.tile_pool(name=f"qp{p}", bufs=3) as qppool,
            tc.tile_pool(name=f"ev{p}", bufs=3) as evpool,
            tc.tile_pool(name=f"id{p}", bufs=1) as idpool,
            tc.tile_pool(name=f"ps{p}", bufs=4, space="PSUM") as pspool,
        ):
            ident = idpool.tile([128, 128], BF16)
            masks.make_identity(nc, ident[:])
            for b in range(2):
                for xc in range(6):
                    wt = wpool.tile([128, 3, 5, 5, 56], BF16)
                    nc.sync.dma_start(
                        out=wt[:],
                        in_=bass.AP(
                            tensor=wdram,
                            offset=(b * 768 + xc * 128) * 75 * 56,
                            ap=[[75 * 56, 128], [25 * 56, 3], [5 * 56, 5], [56, 5], [1, 56]],
                        ),
                    )
                    for r0, nr in CHUNKS[p]:
                        r4 = nr + 4
                        y0 = r0 - 2
                        banks = [(h, min(8, nr - h)) for h in range(0, nr, 8)]
                        ct4 = c4pool.tile([128, r4, 5, 66], BF16)
                        nc.sync.dma_start(
                            out=ct4[:],
                            in_=bass.AP(
                                tensor=src,
                                offset=b * S_B + (r0 - 2) * S_ROW + xc * 128 * S_X,
                                ap=[[S_X, 128], [S_ROW, r4], [66, 5], [1, 66]],
                            ),
                        )
                        ct2 = c2pool.tile([128, 5, 66, r4], BF16)
                        nc.scalar.copy(ct2[:], ct4[:].rearrange("p r j s -> p j s r"))
                        psums = [
                            pspool.tile([128, hh, 64], F32, name=f"ps{p}_{bi}")
                            for bi, (_, hh) in enumerate(banks)
                        ]
                        n_mm = [0] * len(banks)
                        for g, i in DVE_GROUPS:
                            q = qdpool.tile([128, 5, 64, nr], BF16)
                            nc.vector.tensor_mul(
                                out=q[:],
                                in0=ct2[:, :, g : g + 64, i : i + nr],
                                in1=wt[:, g, i, :, y0 : y0 + nr]
                                .unsqueeze(2)
                                .broadcast_to([128, 5, 64, nr]),
                            )
                            for j in range(5):
                                for bi, (h, hh) in enumerate(banks):
                                    nc.tensor.matmul(
                                        psums[bi][:],
                                        ident[:],
                                        q[:, j, :, h : h + hh].rearrange(
                                            "p d r -> p r d"
                                        ),
                                        start=(n_mm[bi] == 0),
                                        stop=(n_mm[bi] == 74),
                                    )
                                    n_mm[bi] += 1
                        for g, i in POOL_GROUPS:
                            qp = qppool.tile([128, nr, 5, 64], BF16)
                            nc.gpsimd.tensor_mul(
                                out=qp[:],
                                in0=ct4[:, i : i + nr, :, g : g + 64],
                                in1=wt[:, g, i, :, y0 : y0 + nr]
                                .rearrange("p j r -> p r j")
                                .unsqueeze(3)
                                .broadcast_to([128, nr, 5, 64]),
                            )
                            for j in range(5):
                                for bi, (h, hh) in enumerate(banks):
                                    nc.tensor.matmul(
                                        psums[bi][:],
                                        ident[:],
                                        qp[:, h : h + hh, j, :],
                                        start=(n_mm[bi] == 0),
                                        stop=(n_mm[bi] == 74),
                                    )
                                    n_mm[bi] += 1
                        ev = evpool.tile([128, nr, 64], ev_dt)
                        for bi, (h, hh) in enumerate(banks):
                            nc.scalar.copy(ev[:, h : h + hh, :], psums[bi][:])
                        if p < 3:
                            dst_ap = bass.AP(
                                tensor=dst,
                                offset=b * S_B
                                + r0 * S_ROW
                                + (xc * 128 + 2) * S_X
                                + 1,
                                ap=[[S_X, 128], [S_ROW, nr], [1, 64]],
                            )
                        else:
                            dst_ap = bass.AP(
                                tensor=dst,
                                offset=b * O_B + (r0 - 6) * O_ROW + xc * 128 * 64,
                                ap=[[64, 128], [O_ROW, nr], [1, 64]],
                            )
                        nc.sync.dma_start(out=dst_ap, in_=ev[:])


def _build():
    nc = bass.Bass()
    a = nc.dram_tensor("a", [2, 60, 772, 66], BF16, kind="ExternalInput")
    w = nc.dram_tensor("w", [2, 768, 75, 56], BF16, kind="ExternalInput")
    bs = nc.dram_tensor("bslab", [2, 60, 772, 66], BF16, kind="Internal")
    cs = nc.dram_tensor("cslab", [2, 60, 772, 66], BF16, kind="Internal")
    o = nc.dram_tensor("o", [2, 48, 768, 64], F32, kind="ExternalOutput")

    # ctx 0: zero both intermediate slabs (borders must read as zero pad).
    # The final 64 elements (b=1, row 59, x=771, d>=2) are never read, so
    # the //128 truncation on the last chunk is harmless.
    total = 2 * 60 * 772 * 66
    with TileContext(nc) as tc:
        with tc.tile_pool(name="z", bufs=1) as zp:
            zt = zp.tile([128, 4096], BF16)
            nc.vector.memset(zt[:], 0.0)
            for dstt in (bs, cs):
                off = 0
                while off < total:
                    n = min(128 * 4096, total - off)
                    cols = n // 128
                    nc.sync.dma_start(
                        out=bass.AP(
                            tensor=dstt, offset=off, ap=[[cols, 128], [1, cols]]
                        ),
                        in_=zt[:, :cols],
                    )
                    off += n

    _emit_pass(nc, a, w, bs, 1)
    _emit_pass(nc, bs, w, cs, 2)
    _emit_pass(nc, cs, w, o, 3)
    _split_waits(nc)
    return nc


def _prep_core(input1, input2, k):
    """Host-side shard prep for core k: bf16 padded cost slab + transposed
    weight slab (zeroed outside the image)."""
    s = k * ROWS
    slab = np.zeros((2, 60, 772, 66), NP_BF16)
    lo, hi = max(0, s - 6), min(H, s + 54)
    slab[:, lo - (s - 6) : hi - (s - 6), 2:770, 1:65] = (
        input1[:, :, lo:hi, :].transpose(0, 2, 3, 1).astype(NP_BF16)
    )
    wsl = np.zeros((2, 768, 75, 56), NP_BF16)
    lo2, hi2 = max(0, s - 4), min(H, s + 52)
    wsl[:, :, :, lo2 - (s - 4) : hi2 - (s - 4)] = (
        input2[:, :, lo2:hi2, :].transpose(0, 3, 1, 2).astype(NP_BF16)
    )
    return {"a": slab, "w": wsl}


_NC_CACHE = [None]


def kernel(input1: np.ndarray, input2: np.ndarray) -> np.ndarray:
    input1 = np.asarray(input1, dtype=np.float32)
    input2 = np.asarray(input2, dtype=np.float32)
    if _NC_CACHE[0] is None:
        _NC_CACHE[0] = _build()
    nc = _NC_CACHE[0]

    in_maps = [_prep_core(input1, input2, k) for k in range(N_CORES)]

    # trace=False: the axon NTFF profile hook (antenv.axon_hooks) is not
    # available in this container and trace=True fails before compiling.
    res = bass_utils.run_bass_kernel_spmd(
        nc, in_maps, core_ids=list(range(N_CORES)), trace=False
    )
    LAST_EXEC_NS[0] = res.exec_time_ns

    out = np.empty((B, D, H, W), np.float32)
    for k in range(N_CORES):
        s = k * ROWS
        out[:, :, s : s + ROWS, :] = res.results[k]["o"].transpose(0, 3, 1, 2)
    return out


# revision 12
# speedup vs baseline: 1.2022x; 1.1793x over previous
"""LGA3 (3x local guided aggregation, radius 2) on 8 TRN2 NeuronCores.

Sharding: H split 8 ways (48 output rows/core), redundant-halo compute
(pass1 computes 56 rows, pass2 52, pass3 48) -> no inter-core comms.

v2 multi-engine design (vs v1 DVE-only fp32):
- bf16 data everywhere; PSUM accumulates in fp32 (exact).
- Per 8-row chunk and tap-group (g,i) of 5 j-taps:
    DVE (12 groups): q = window * w  -- one tensor_tensor in 2x perf mode
      (all operands bf16, innermost stride-1 = row dim -> needs the
      row-innermost "transposed" cost tile ct2 made by the ACT engine).
    Pool (3 groups): same on the row-major tile ct4 (no perf modes on
      Pool so strides don't matter).
    PE: 75 identity-stationary matmuls accumulate all q tiles into one
      PSUM bank (fp32, free).  ACT evicts PSUM -> bf16 slab (fp32 final).
- Layout: partition = x (128-chunks); cost slab [b, y, x, d] with x
  padded +-2 (772) and d padded +-1 (66) so every (j, g) shift is an
  affine free-dim offset; the 5 x-shifts are the j dim (stride 66).
  Weights pre-transposed on host to [b, x, t, y] (y innermost) with
  zeroed out-of-image rows.
"""

import os
import sys

for _p in ("/opt/trn_rl_repo", "/root/.axon_site/_ro/trn_rl_repo"):
    if os.path.isdir(_p) and _p not in sys.path:
        sys.path.append(_p)

import numpy as np
import concourse.bass as bass
import concourse.mybir as mybir
from concourse.tile import TileContext
from concourse import bass_utils, masks

F32 = mybir.dt.float32
BF16 = mybir.dt.bfloat16
NP_BF16 = mybir.dt.np(BF16)

B, D, H, W = 2, 64, 384, 768
N_CORES = 8
ROWS = H // N_CORES  # 48 output rows per core

# cost slab [b, 60 rows, 772 x, 66 d] (strides in elements)
S_X, S_ROW = 66, 772 * 66
S_B = 60 * S_ROW
# weight dram [b, 768 x, 75 t, 56 y]
W_B = 768 * 75 * 56
# output [b, 48 rows, 768 x, 64 d]
O_ROW, O_B = 768 * 64, 48 * 768 * 64

# chunk lists per pass: (slab row of first output row, n output rows).
# 16-row blocks amortize per-instruction and per-chunk fixed costs; each
# block accumulates into ceil(nr/8) PSUM banks (a bank holds 8x64 fp32).
CHUNKS = {
    1: [(2, 16), (18, 16), (34, 16), (50, 8)],   # rows 2..57
    2: [(4, 16), (20, 16), (36, 16), (52, 4)],   # rows 4..55
    3: [(6, 16), (22, 16), (38, 16)],            # rows 6..53
}

GROUPS = [(g, i) for g in range(3) for i in range(5)]
POOL_GROUPS = [(0, 0), (1, 2), (2, 4)]
# DVE takes the other 12 (g,i) groups.  (Merging i-pairs into one
# instruction needs a 4th free dim, which the TensorTensor ISA pattern
# rejects: "Expected 3D tensor for ISA mem pattern".)
DVE_GROUPS = [gi for gi in GROUPS if gi not in POOL_GROUPS]

LAST_EXEC_NS = [None]


def _split_waits(nc, max_waits=1):
    """Split >max_waits sync waits on one instruction into preceding
    wait-only drains (walrus setupSyncWait limit workaround)."""
    ctr = [0]
    for f in nc.m.functions:
        for blk in f.blocks:
            new_list = []
            for inst in blk.instructions:
                si = getattr(inst, "sync_info", None)
                if si is not None and si.on_wait and len(si.on_wait) > max_waits:
                    waits = list(si.on_wait)
                    extra, keep = waits[:-max_waits], waits[-max_waits:]
                    for wcond in extra:
                        ctr[0] += 1
                        nop = mybir.InstDrain(
                            name=f"waitsplit_{ctr[0]}", ins=[], outs=[]
                        )
                        nop.engine = inst.engine
                        nop.sync_info = mybir.SyncInfo(on_wait=[wcond], on_update=[])
                        new_list.append(nop)
                        nc.register_instruction(nop, overwrite=True)
                    si.on_wait = keep
                new_list.append(inst)
            blk.instructions = new_list
    return nc


def _emit_pass(nc, src, wdram, dst, p):
    """One LGA pass: src slab -> dst (slab for p<3, compact output p=3)."""
    ev_dt = F32 if p == 3 else BF16
    with TileContext(nc) as tc:
        with (
            tc.tile_pool(name=f"w{p}", bufs=2) as wpool,
            tc.tile_pool(name=f"c4{p}", bufs=3) as c4pool,
            tc.tile_pool(name=f"c2{p}", bufs=3) as c2pool,
            tc.tile_pool(name=f"qd{p}", bufs=4) as qdpool,
            tc.tile_pool(name=f"qp{p}", bufs=3) as qppool,
            tc.tile_pool(name=f"ev{p}", bufs=3) as evpool,
            tc.tile_pool(name=f"id{p}", bufs=1) as idpool,
            tc.tile_pool(name=f"ps{p}", bufs=4, space="PSUM") as pspool,
        ):
            ident = idpool.tile([128, 128], BF16)
            masks.make_identity(nc, ident[:])
            for b in range(2):
                for xc in range(6):
                    wt = wpool.tile([128, 3, 5, 5, 56], BF16)
                    nc.sync.dma_start(
                        out=wt[:],
                        in_=bass.AP(
                            tensor=wdram,
                            offset=(b * 768 + xc * 128) * 75 * 56,
                            ap=[[75 * 56, 128], [25 * 56, 3], [5 * 56, 5], [56, 5], [1, 56]],
                        ),
                    )
                    for r0, nr in CHUNKS[p]:
                        r4 = nr + 4
                        y0 = r0 - 2
                        banks = [(h, min(8, nr - h)) for h in range(0, nr, 8)]
                        ct4 = c4pool.tile([128, r4, 5, 66], BF16)
                        nc.sync.dma_start(
                            out=ct4[:],
                            in_=bass.AP(
                                tensor=src,
                                offset=b * S_B + (r0 - 2) * S_ROW + xc * 128 * S_X,
                                ap=[[S_X, 128], [S_ROW, r4], [66, 5], [1, 66]],
                            ),
                        )
                        ct2 = c2pool.tile([128, 5, 66, r4], BF16)
                        nc.scalar.copy(ct2[:], ct4[:].rearrange("p r j s -> p j s r"))
                        psums = [
                            pspool.tile([128, hh, 64], F32, name=f"ps{p}_{bi}")
                            for bi, (_, hh) in enumerate(banks)
                        ]
                        n_mm = [0] * len(banks)
                        for g, i in DVE_GROUPS:
                            q = qdpool.tile([128, 5, 64, nr], BF16)
                            nc.vector.tensor_mul(
                                out=q[:],
                                in0=ct2[:, :, g : g + 64, i : i + nr],
                                in1=wt[:, g, i, :, y0 : y0 + nr]
                                .unsqueeze(2)
                                .broadcast_to([128, 5, 64, nr]),
                            )
                            for j in range(5):
                                for bi, (h, hh) in enumerate(banks):
                                    nc.tensor.matmul(
                                        psums[bi][:],
                                        ident[:],
                                        q[:, j, :, h : h + hh].rearrange(
                                            "p d r -> p r d"
                                        ),
                                        start=(n_mm[bi] == 0),
                                        stop=(n_mm[bi] == 74),
                                    )
                                    n_mm[bi] += 1
                        for g, i in POOL_GROUPS:
                            qp = qppool.tile([128, nr, 5, 64], BF16)
                            nc.gpsimd.tensor_mul(
                                out=qp[:],
                                in0=ct4[:, i : i + nr, :, g : g + 64],
                                in1=wt[:, g, i, :, y0 : y0 + nr]
                                .rearrange("p j r -> p r j")
                                .unsqueeze(3)
                                .broadcast_to([128, nr, 5, 64]),
                            )
                            for j in range(5):
                                for bi, (h, hh) in enumerate(banks):
                                    nc.tensor.matmul(
                                        psums[bi][:],
                                        ident[:],
                                        qp[:, h : h + hh, j, :],
                                        start=(n_mm[bi] == 0),
                                        stop=(n_mm[bi] == 74),
                                    )
                                    n_mm[bi] += 1
                        ev = evpool.tile([128, nr, 64], ev_dt)
                        for bi, (h, hh) in enumerate(banks):
                            nc.scalar.copy(ev[:, h : h + hh, :], psums[bi][:])
                        if p < 3:
                            dst_ap = bass.AP(
                                tensor=dst,
                                offset=b * S_B
                                + r0 * S_ROW
                                + (xc * 128 + 2) * S_X
                                + 1,
                                ap=[[S_X, 128], [S_ROW, nr], [1, 64]],
                            )
                        else:
                            dst_ap = bass.AP(
                                tensor=dst,
                                offset=b * O_B + (r0 - 6) * O_ROW + xc * 128 * 64,
                                ap=[[64, 128], [O_ROW, nr], [1, 64]],
                            )
                        nc.sync.dma_start(out=dst_ap, in_=ev[:])


def _build():
    nc = bass.Bass()
    a = nc.dram_tensor("a", [2, 60, 772, 66], BF16, kind="ExternalInput")
    w = nc.dram_tensor("w", [2, 768, 75, 56], BF16, kind="ExternalInput")
    bs = nc.dram_tensor("bslab", [2, 60, 772, 66], BF16, kind="Internal")
    cs = nc.dram_tensor("cslab", [2, 60, 772, 66], BF16, kind="Internal")
    o = nc.dram_tensor("o", [2, 48, 768, 64], F32, kind="ExternalOutput")

    # ctx 0: zero both intermediate slabs (borders must read as zero pad).
    # The final 64 elements (b=1, row 59, x=771, d>=2) are never read, so
    # the //128 truncation on the last chunk is harmless.
    total = 2 * 60 * 772 * 66
    with TileContext(nc) as tc:
        with tc.tile_pool(name="z", bufs=1) as zp:
            zt = zp.tile([128, 4096], BF16)
            nc.vector.memset(zt[:], 0.0)
            for dstt in (bs, cs):
                off = 0
                while off < total:
                    n = min(128 * 4096, total - off)
                    cols = n // 128
                    nc.sync.dma_start(
                        out=bass.AP(
                            tensor=dstt, offset=off, ap=[[cols, 128], [1, cols]]
                        ),
                        in_=zt[:, :cols],
                    )
                    off += n

    _emit_pass(nc, a, w, bs, 1)
    _emit_pass(nc, bs, w, cs, 2)
    _emit_pass(nc, cs, w, o, 3)
    _split_waits(nc)
    return nc


def _prep_full(input1, input2):
    """One-shot host prep shared by all cores: full padded bf16 cost slab
    (row axis = input rows -6..389) and transposed weight slab (y axis =
    input rows -4..387, zeroed outside the image)."""
    fullc = np.zeros((2, H + 12, 772, 66), NP_BF16)
    fullc[:, 6 : H + 6, 2:770, 1:65] = input1.transpose(0, 2, 3, 1)
    fullw = np.zeros((2, 768, 75, H + 8), NP_BF16)
    fullw[:, :, :, 4 : H + 4] = input2.transpose(0, 3, 1, 2)
    return fullc, fullw


def _prep_core(input1, input2, k, _full=None):
    """Host-side shard prep for core k (slices the shared full slabs)."""
    fullc, fullw = _full if _full is not None else _prep_full(input1, input2)
    s = k * ROWS
    return {
        "a": np.ascontiguousarray(fullc[:, s : s + 60]),
        "w": np.ascontiguousarray(fullw[:, :, :, s : s + 56]),
    }


_NC_CACHE = [None]


def kernel(input1: np.ndarray, input2: np.ndarray) -> np.ndarray:
    input1 = np.asarray(input1, dtype=np.float32)
    input2 = np.asarray(input2, dtype=np.float32)
    if _NC_CACHE[0] is None:
        _NC_CACHE[0] = _build()
    nc = _NC_CACHE[0]

    full = _prep_full(input1, input2)
    in_maps = [_prep_core(input1, input2, k, _full=full) for k in range(N_CORES)]

    # trace=False: the axon NTFF profile hook (antenv.axon_hooks) is not
    # available in this container and trace=True fails before compiling.
    res = bass_utils.run_bass_kernel_spmd(
        nc, in_maps, core_ids=list(range(N_CORES)), trace=False
    )
    LAST_EXEC_NS[0] = res.exec_time_ns

    out = np.empty((B, D, H, W), np.float32)
    for k in range(N_CORES):
        s = k * ROWS
        out[:, :, s : s + ROWS, :] = res.results[k]["o"].transpose(0, 3, 1, 2)
    return out
